# revision 1
# baseline (speedup 1.0000x reference)
"""Bayesian transformer block on 8 trn2 cores — v2.

Sharding: core c -> batch b=c//2, half h=c%2.  Queries are 8 interleaved
256-row chunks (chunk c covers rows [512c+256h, 512c+256h+256)), processed
as 4 chunk-PAIRS of 512 queries (pair t = chunks 2t, 2t+1) so every
matmul's moving free dim is 512.  Pair t attends keys [0, 1024(t+1)) —
static across cores; causality inside the trailing 1024 keys is enforced
by a data-driven multiplicative mask (8 j-tiles, input `cmask`).

All matmul operands live in "paired" layout [128, nblk, free] so each
stage can run as 2 bf16 matmuls (slices [:, b, :]) or one fp8e4
DoubleRow matmul ([:, b:b+2, :]) — per-stage dtype switches below.

Full K^T / V for all 4096 keys is cached in SBUF (no DRAM spill).
Attention uses transposed scores S^T[k,q]; exp output feeds P@V directly
as the moving operand; row sums via an all-ones stationary matmul;
normalization is a reciprocal broadcast multiply before the output
projection.  PSUM: psA 3 banks (scores/wo/ffn1/transposes), psAV 4 banks
(AV accum, then ffn2 accum), psS 1 bank (row sums).
"""
import sys, os

for _p in ("/opt/trn_rl_repo", "/root/.axon_site/_ro/trn_rl_repo"):
    if os.path.isdir(_p) and _p not in sys.path:
        sys.path.insert(0, _p)

import numpy as np
from contextlib import ExitStack

import concourse.bass as bass
import concourse.bacc as bacc
import concourse.mybir as mybir
import concourse.tile as tile
from concourse.bass_utils import run_bass_kernel_spmd
from concourse.masks import make_identity

F32 = mybir.dt.float32
BF16 = mybir.dt.bfloat16
F8 = mybir.dt.float8e4
AF = mybir.ActivationFunctionType
OP = mybir.AluOpType
DR = mybir.MatmulPerfMode.DoubleRow
ts = bass.ts

DIM = 512
HID = 2048
BS, SLEN = 4, 4096
NG = SLEN // 512
NPAIR = 4
NQROWS = 2048
INV_SQRT_D = float(1.0 / np.sqrt(DIM))

# --- per-stage matmul dtypes (BF16, or F8 => fp8e4 + DoubleRow) ---
DT_X = BF16    # x^T/xq^T tiles, wkT/wvT, K^T cache  (kv-proj + scores)
DT_P = BF16    # attn probs, V cache, ones, mask     (attn @ V)
DT_H = BF16    # normalized h, woT                   (output proj)
DT_F = BF16    # h_resT, w1T                         (ffn1)
DT_G = BF16    # relu(f1), w2T                       (ffn2)
EXP_BIAS = 0.0

_CACHE = {}


def _build_nc(reps=1):
    nc = bacc.Bacc("TRN2", target_bir_lowering=False, debug=False, num_devices=8,
                   dynamic_dma_scratch_size=2048)

    xf = nc.dram_tensor("xf", [SLEN, DIM], F32, kind="ExternalInput").ap()
    xq = nc.dram_tensor("xq", [NQROWS, DIM], F32, kind="ExternalInput").ap()
    cmask = nc.dram_tensor("cmask", [8, 128, 512], F32, kind="ExternalInput").ap()
    wio = {}
    for w, (o, i) in (("wk", (DIM, DIM)), ("wv", (DIM, DIM)), ("wo", (DIM, DIM)),
                      ("w1", (HID, DIM)), ("w2", (DIM, HID))):
        for sfx in ("mu", "ls", "eps"):
            wio[f"{w}_{sfx}"] = nc.dram_tensor(f"{w}_{sfx}", [o, i], F32,
                                               kind="ExternalInput").ap()
    out = nc.dram_tensor("out", [NQROWS, DIM], F32, kind="ExternalOutput").ap()

    with tile.TileContext(nc) as tc:
      for _rep in range(reps):
       with ExitStack() as ctx:
        P = lambda n: f"{n}_{_rep}"
        const = ctx.enter_context(tc.tile_pool(name=P("const"), bufs=1))
        kvc = ctx.enter_context(tc.tile_pool(name=P("kvc"), bufs=1))
        xqp = ctx.enter_context(tc.tile_pool(name=P("xqp"), bufs=1))
        wres = ctx.enter_context(tc.tile_pool(name=P("wres"), bufs=1))
        work = ctx.enter_context(tc.tile_pool(name=P("work"), bufs=1))
        psA = ctx.enter_context(tc.tile_pool(name=P("psA"), bufs=3, space="PSUM"))
        psAV = ctx.enter_context(tc.tile_pool(name=P("psAV"), bufs=1, space="PSUM"))
        psS = ctx.enter_context(tc.tile_pool(name=P("psS"), bufs=1, space="PSUM"))

        # ---------------- constants ----------------
        need_f8 = F8 in (DT_X, DT_P, DT_H, DT_F, DT_G)
        with ExitStack() as stk:
            cstg = stk.enter_context(tc.tile_pool(name=P("cstg"), bufs=1))
            identf = cstg.tile([128, 128], F32, tag="identf")
            make_identity(nc, identf[:])
            ident = const.tile([128, 128], BF16, tag="ident")
            nc.vector.tensor_copy(ident[:], identf[:])
            ident8 = None
            if need_f8:
                ident8 = const.tile([128, 128], F8, tag="ident8")
                nc.vector.tensor_copy(ident8[:], identf[:])
            ones32 = cstg.tile([128, 2, 128], F32, tag="ones32")
            nc.gpsimd.memset(ones32[:], 1.0)
            ones = const.tile([128, 2, 128], DT_P, tag="ones")
            nc.vector.tensor_copy(ones[:], ones32[:])
            cmf = cstg.tile([128, 8, 512], F32, tag="cmf")
            nc.sync.dma_start(cmf[:], cmask.rearrange("j p q -> p j q"))
            cm = const.tile([128, 8, 512], DT_P, tag="cm")
            nc.vector.tensor_copy(cm[:], cmf[:])

        def tident(dt):
            return ident8 if dt == F8 else ident

        def tp4(ps, src, dt, n=4):
            """n batched 128x128 transposes of src[:, 128*i:128*(i+1)] into
            ps[:, i, :] (single PSUM bank)."""
            for i in range(n):
                nc.tensor.matmul(ps[:, i, :], src[:, ts(i, 128)], tident(dt)[:],
                                 is_transpose=True, start=(i == 0),
                                 stop=(i == n - 1), skip_group_check=True)

        def pmm(out_ap, lhs_fn, rhs_fn, dt, start, stop, npair=2, sgc=False):
            """Accumulating matmul chain over `npair` 256-deep contraction
            pairs.  lhs_fn/rhs_fn(pair_idx, b) -> AP; b=None => DoubleRow."""
            if dt == F8:
                for p in range(npair):
                    nc.tensor.matmul(out_ap, lhs_fn(p, None), rhs_fn(p, None),
                                     start=start and p == 0,
                                     stop=stop and p == npair - 1,
                                     perf_mode=DR, skip_group_check=sgc)
            else:
                k = 0
                for p in range(npair):
                    for b in range(2):
                        nc.tensor.matmul(out_ap, lhs_fn(p, b), rhs_fn(p, b),
                                         start=start and k == 0,
                                         stop=stop and k == 2 * npair - 1,
                                         skip_group_check=sgc)
                        k += 1

        # -------- weight build: W = mu + exp(ls)*eps, transposed+paired ----
        def build_wT(w, o_dim, i_dim, dt):
            """Tile [128, i_dim//128, o_dim] dtype dt holding W^T
            (partition = input-dim within block, dim1 = input block,
            free = output dim)."""
            wt = wres.tile([128, i_dim // 128, o_dim], dt, tag=f"{w}T",
                           name=f"{w}T")
            mu_r = wio[f"{w}_mu"].rearrange("(a p) i -> a p i", p=128)
            ls_r = wio[f"{w}_ls"].rearrange("(a p) i -> a p i", p=128)
            ep_r = wio[f"{w}_eps"].rearrange("(a p) i -> a p i", p=128)
            IC = min(i_dim, 512)
            with ExitStack() as stk:
                stage = stk.enter_context(tc.tile_pool(name=P(f"stg_{w}"), bufs=2))
                for a in range(o_dim // 128):
                    for cb in range(i_dim // IC):
                        mu = stage.tile([128, IC], F32, tag="mu")
                        ls = stage.tile([128, IC], F32, tag="ls")
                        ep = stage.tile([128, IC], F32, tag="ep")
                        nc.sync.dma_start(mu[:], mu_r[a][:, ts(cb, IC)])
                        nc.sync.dma_start(ls[:], ls_r[a][:, ts(cb, IC)])
                        nc.sync.dma_start(ep[:], ep_r[a][:, ts(cb, IC)])
                        els = stage.tile([128, IC], F32, tag="els")
                        nc.scalar.activation(els[:], ls[:], AF.Exp)
                        prod = stage.tile([128, IC], F32, tag="prod")
                        nc.gpsimd.tensor_tensor(prod[:], els[:], ep[:], op=OP.mult)
                        wnat = stage.tile([128, IC], dt, tag="wnat")
                        nc.vector.tensor_tensor(wnat[:], prod[:], mu[:], op=OP.add)
                        nb = IC // 128
                        ps = psA.tile([128, nb, 128], dt, tag="A")
                        tp4(ps, wnat, dt, n=nb)
                        nc.vector.tensor_copy(
                            wt[:, ts(cb, nb), ts(a, 128)], ps[:])
            return wt

        # ---- xq^T for all 4 pairs first: gives the PE transpose work ----
        # (overlapping the wk/wv weight DMA+build that has no PE component)
        xq_r = xq.rearrange("(t j q) d -> t j q d", j=4, q=128)
        xqTs = {}
        for t in (3, 2, 1, 0):
            xqT = xqp.tile([128, 4, 512], DT_X, tag=f"xqT{t}", name=f"xqT{t}")
            for j in range(4):
                xn = work.tile([128, DIM], F32, tag="qxn", bufs=2)
                nc.sync.dma_start(xn[:], xq_r[t, j])
                xb = work.tile([128, DIM], DT_X, tag="qxb", bufs=2)
                nc.scalar.copy(xb[:], xn[:])
                ps = psA.tile([128, 4, 128], DT_X, tag="A")
                tp4(ps, xb, DT_X)
                nc.vector.tensor_copy(xqT[:, :, ts(j, 128)], ps[:])
            xqTs[t] = xqT

        wkT = build_wT("wk", DIM, DIM, DT_X)
        wvT = build_wT("wv", DIM, DIM, DT_P)

        # ---------------- K^T / V over all 8 groups ----------------
        # phase A: transpose all of x^T into resident tiles (PE work that
        # needs no weights); phase B: dense K/V matmul stream.
        xf_r = xf.rearrange("(g j p) d -> g j p d", j=4, p=128)
        kt = kvc.tile([128, 4, SLEN], DT_X, tag="kt", name="kt")
        vp = [kvc.tile([128, 4, DIM], DT_P, tag=f"vp{g}", name=f"vp{g}")
              for g in range(NG)]

        with ExitStack() as stk:
            xstg = stk.enter_context(tc.tile_pool(name=P("xstg"), bufs=3))
            for wave in range(2):
                xfTs = {}
                for g in range(4 * wave, 4 * wave + 4):
                    xfT = xstg.tile([128, 4, 512], DT_X, tag=f"xfT{g % 4}",
                                    name=f"xfT{g % 4}", bufs=1)
                    for j in range(4):
                        xn = xstg.tile([128, DIM], F32, tag="xn")
                        nc.sync.dma_start(xn[:], xf_r[g, j])
                        xb = xstg.tile([128, DIM], DT_X, tag="xb")
                        nc.scalar.copy(xb[:], xn[:])
                        ps = psA.tile([128, 4, 128], DT_X, tag="A")
                        tp4(ps, xb, DT_X)
                        nc.vector.tensor_copy(xfT[:, :, ts(j, 128)], ps[:])
                    xfTs[g] = xfT
                for g in range(4 * wave, 4 * wave + 4):
                    xfT = xfTs[g]
                    for o in range(4):
                        pk = psAV.tile([128, 512], F32, tag=f"av{o}")
                        pmm(pk[:],
                            lambda p, b, o=o: wkT[:, ts(p, 2), ts(o, 128)]
                            if b is None else wkT[:, 2 * p + b, ts(o, 128)],
                            lambda p, b, xfT=xfT: xfT[:, ts(p, 2), :]
                            if b is None else xfT[:, 2 * p + b, :],
                            DT_X, True, True)
                        nc.vector.tensor_copy(kt[:, o, ts(g, 512)], pk[:])
                    for j in range(4):
                        pv = psAV.tile([128, 512], F32, tag=f"av{j}")
                        pmm(pv[:],
                            lambda p, b, j=j, xfT=xfT: xfT[:, ts(p, 2), ts(j, 128)]
                            if b is None else xfT[:, 2 * p + b, ts(j, 128)],
                            lambda p, b: wvT[:, ts(p, 2), :]
                            if b is None else wvT[:, 2 * p + b, :],
                            DT_X, True, True)
                        nc.scalar.copy(vp[g][:, j, :], pv[:])

        woT = build_wT("wo", DIM, DIM, DT_H)
        w1T = build_wT("w1", HID, DIM, DT_F)
        w2T = build_wT("w2", DIM, HID, DT_G)

        # ---------------- per-pair attention + FFN ----------------
        out_r = out.rearrange("(t qs q) d -> t qs q d", qs=4, q=128)

        for t in (3, 2, 1, 0):
            xqT = xqTs[t]

            hAV = [psAV.tile([128, 512], F32, tag=f"av{i}", name=f"hAV{i}")
                   for i in range(4)]
            srep = psS.tile([128, 512], F32, tag="s", name="srep")
            # interior groups: full 512-query blocks, no mask
            for g in range(2 * t):
                ptv = work.tile([128, 4, 512], DT_P, tag="pt", bufs=2)
                for j in range(4):
                    sA = psA.tile([128, 512], F32, tag="A")
                    pmm(sA[:],
                        lambda p, b, kx=4 * g + j: kt[:, ts(p, 2), ts(kx, 128)]
                        if b is None else kt[:, 2 * p + b, ts(kx, 128)],
                        lambda p, b: xqT[:, ts(p, 2), :]
                        if b is None else xqT[:, 2 * p + b, :],
                        DT_X, True, True)
                    nc.scalar.activation(ptv[:, j, :], sA[:], AF.Exp,
                                         scale=INV_SQRT_D, bias=EXP_BIAS)
                    if j % 2 == 1:
                        jp = j // 2
                        first = (g == 0 and jp == 0)
                        for i in range(4):
                            pmm(hAV[i][:],
                                lambda p, b, i=i, jp=jp, g=g:
                                vp[g][:, ts(jp, 2), ts(i, 128)]
                                if b is None else vp[g][:, 2 * jp + b, ts(i, 128)],
                                lambda p, b, jp=jp: ptv[:, ts(jp, 2), :]
                                if b is None else ptv[:, 2 * jp + b, :],
                                DT_P, first, False, npair=1, sgc=True)
                        pmm(srep[:],
                            lambda p, b: ones[:, :, :] if b is None
                            else ones[:, b, :],
                            lambda p, b, jp=jp: ptv[:, ts(jp, 2), :]
                            if b is None else ptv[:, 2 * jp + b, :],
                            DT_P, first, False, npair=1, sgc=True)
            # diagonal: 3 half-blocks of 256 queries (one fully-masked
            # half-block of group 2t+1 is skipped entirely).  The mask for
            # every masked half-block is cm[:, j, 0:256].
            started = 2 * t > 0
            for dg, qh, masked, dstop in ((2 * t, 0, True, True),
                                          (2 * t, 1, False, False),
                                          (2 * t + 1, 1, True, True)):
                ptd = work.tile([128, 4, 256], DT_P, tag="ptd", bufs=2)
                for j in range(4):
                    sA = psA.tile([128, 256], F32, tag="A")
                    pmm(sA[:],
                        lambda p, b, kx=4 * dg + j: kt[:, ts(p, 2), ts(kx, 128)]
                        if b is None else kt[:, 2 * p + b, ts(kx, 128)],
                        lambda p, b, qh=qh: xqT[:, ts(p, 2), ts(qh, 256)]
                        if b is None else xqT[:, 2 * p + b, ts(qh, 256)],
                        DT_X, True, True)
                    if masked:
                        pe = work.tile([128, 256], DT_P, tag="ped", bufs=2)
                        nc.scalar.activation(pe[:], sA[:], AF.Exp,
                                             scale=INV_SQRT_D, bias=EXP_BIAS)
                        nc.vector.tensor_tensor(ptd[:, j, :], pe[:],
                                                cm[:, j, ts(0, 256)],
                                                op=OP.mult)
                    else:
                        nc.scalar.activation(ptd[:, j, :], sA[:], AF.Exp,
                                             scale=INV_SQRT_D, bias=EXP_BIAS)
                    if j % 2 == 1:
                        jp = j // 2
                        first = (not started) and jp == 0
                        last = dstop and jp == 1
                        for i in range(4):
                            pmm(hAV[i][:, ts(qh, 256)],
                                lambda p, b, i=i, jp=jp, dg=dg:
                                vp[dg][:, ts(jp, 2), ts(i, 128)]
                                if b is None else vp[dg][:, 2 * jp + b, ts(i, 128)],
                                lambda p, b, jp=jp: ptd[:, ts(jp, 2), :]
                                if b is None else ptd[:, 2 * jp + b, :],
                                DT_P, first, last, npair=1, sgc=True)
                        pmm(srep[:, ts(qh, 256)],
                            lambda p, b: ones[:, :, :] if b is None
                            else ones[:, b, :],
                            lambda p, b, jp=jp: ptd[:, ts(jp, 2), :]
                            if b is None else ptd[:, 2 * jp + b, :],
                            DT_P, first, last, npair=1, sgc=True)
                if qh == 1:
                    started = True

            # output projection on UNNORMALIZED h, normalization deferred
            # past the linear wo (wo(h)/s == wo(h/s)): the reciprocal runs
            # on DVE in parallel with the wo matmuls instead of blocking
            # them, and the PSUM->SBUF feed copies go to the scalar engine.
            r_bc = work.tile([128, 512], F32, tag="r", bufs=2)
            nc.vector.reciprocal(r_bc[:], srep[:])
            h_nrm = work.tile([128, 4, 512], DT_H, tag="hn", bufs=2)
            for i in range(4):
                nc.scalar.copy(h_nrm[:, i, :], hAV[i][:])
            h_resT = work.tile([128, 4, 512], DT_F, tag="hr", bufs=2)
            for o in range(4):
                wA = psA.tile([128, 512], F32, tag="A")
                pmm(wA[:],
                    lambda p, b, o=o: woT[:, ts(p, 2), ts(o, 128)]
                    if b is None else woT[:, 2 * p + b, ts(o, 128)],
                    lambda p, b: h_nrm[:, ts(p, 2), :]
                    if b is None else h_nrm[:, 2 * p + b, :],
                    DT_H, True, True)
                wr = work.tile([128, 512], F32, tag="wr", bufs=2)
                nc.vector.tensor_tensor(wr[:], wA[:], r_bc[:], op=OP.mult)
                nc.vector.tensor_tensor(h_resT[:, o, :], wr[:], xqT[:, o, :],
                                        op=OP.add)

            # h_res natural orientation (for the final residual add)
            h_resN = [work.tile([128, 512], BF16, tag=f"hN{qs}", name=f"hN{qs}")
                      for qs in range(4)]
            for qs in range(4):
                psn = psA.tile([128, 4, 128], DT_F, tag="A")
                for db in range(4):
                    nc.tensor.matmul(psn[:, db, :], h_resT[:, db, ts(qs, 128)],
                                     tident(DT_F)[:], is_transpose=True,
                                     start=(db == 0), stop=(db == 3),
                                     skip_group_check=True)
                nc.vector.tensor_copy(h_resN[qs][:], psn[:])

            # FFN
            hF2 = [psAV.tile([128, 512], F32, tag=f"av{qs}", name=f"hF2{qs}")
                   for qs in range(4)]
            for hh in range(HID // 128):
                fA = psA.tile([128, 512], F32, tag="A")
                pmm(fA[:],
                    lambda p, b, hh=hh: w1T[:, ts(p, 2), ts(hh, 128)]
                    if b is None else w1T[:, 2 * p + b, ts(hh, 128)],
                    lambda p, b: h_resT[:, ts(p, 2), :]
                    if b is None else h_resT[:, 2 * p + b, :],
                    DT_F, True, True)
                if hh % 2 == 0:
                    f1 = work.tile([128, 2, 512], DT_G, tag="f1", bufs=2)
                nc.scalar.activation(f1[:, hh % 2, :], fA[:], AF.Relu)
                if hh % 2 == 1:
                    hp = hh // 2
                    for qs in range(4):
                        pmm(hF2[qs][:],
                            lambda p, b, qs=qs: f1[:, :, ts(qs, 128)]
                            if b is None else f1[:, b, ts(qs, 128)],
                            lambda p, b, hp=hp: w2T[:, ts(hp, 2), :]
                            if b is None else w2T[:, 2 * hp + b, :],
                            DT_G, hp == 0, hp == 7, npair=1)

            for qs in range(4):
                ot = work.tile([128, 512], F32, tag="ot", bufs=2)
                nc.vector.tensor_tensor(ot[:], hF2[qs][:], h_resN[qs][:],
                                        op=OP.add)
                nc.sync.dma_start(out_r[t, qs], ot[:])

    nc.compile()
    return nc


def _shard_inputs(inputs):
    x = np.ascontiguousarray(inputs["x"], dtype=np.float32)
    # causal mask for the trailing 2 groups of each pair, per half h
    qg = (np.arange(512)[None, None, :] % 256) + 512 * (np.arange(512)[None, None, :] // 256)
    kk = np.arange(128)[None, :, None]
    jm = np.arange(8)[:, None, None]
    in_maps = []
    for c in range(8):
        b, h = c // 2, c % 2
        xb = np.ascontiguousarray(x[b])
        xqh = np.ascontiguousarray(
            xb.reshape(4, 2, 2, 2, 128, DIM)[:, :, h].reshape(NQROWS, DIM))
        cmsk = ((qg + 256 * h) >= (128 * jm + kk)).astype(np.float32)
        m = {"xf": xb, "xq": xqh, "cmask": np.ascontiguousarray(cmsk)}
        for k, v in inputs.items():
            if k not in ("x", "mask"):
                m[k] = np.ascontiguousarray(v, dtype=np.float32)
        in_maps.append(m)
    return in_maps


def kernel(**inputs):
    if "nc" not in _CACHE:
        _CACHE["nc"] = _build_nc()
    nc = _CACHE["nc"]
    in_maps = _shard_inputs(inputs)
    res = run_bass_kernel_spmd(nc, in_maps, core_ids=list(range(8)))
    out = np.empty((BS, SLEN, DIM), dtype=np.float32)
    for c in range(8):
        b, h = c // 2, c % 2
        o = res.results[c]["out"].reshape(4, 2, 2, 128, DIM)
        out.reshape(BS, 4, 2, 2, 2, 128, DIM)[b, :, :, h] = o
    return out

